# revision 1
# baseline (speedup 1.0000x reference)
"""AxialSpaceTimeTransformer fully fused in ONE Bass program on 8 TRN2 cores.

Sharding (8-way, single chip):
  * t-domain: core c holds frames t in [4c, 4c+4) for both batches.
    Space-attention (over s) and FF are core-local here.
  * s-domain: core c holds spatial positions s in [32c, 32c+32).
    Causal time-attention (over t) is core-local here.

The ENTIRE model runs as a single bass_exec custom call per core:
  pre (rv = rmsnorm(tok) @ vrW)  ->  3 space layers  ->  AllToAll(x+rv)
  -> time layer 3 -> AllToAll -> 3 space layers -> AllToAll -> time layer 7
  -> final rmsnorm.  Collectives are in-kernel gpsimd AllToAlls over
  internal DRAM bounce buffers; x stays SBUF-resident between layers.

Precision: residual stream, rv, norms and softmax in fp32; projection /
FF weights and activations in bf16 (PSUM accumulation in fp32); the
score path (tanh softcap logits) in f32r to keep exponent error tiny.
"""

import os
import sys
import types

import numpy as np

if "/opt/trn_rl_repo" not in sys.path:
    sys.path.insert(0, "/opt/trn_rl_repo")

# -- antenv.axon_hooks shim (agent image lacks it; bass_utils wants it) --
import antenv  # noqa: E402

if not hasattr(antenv, "axon_hooks"):
    _hooks = types.ModuleType("antenv.axon_hooks")
    _hooks._hook = None
    _hooks.set_axon_ntff_profile_hook = lambda h: setattr(_hooks, "_hook", h)
    _hooks.get_axon_ntff_profile_hook = lambda: _hooks._hook
    sys.modules["antenv.axon_hooks"] = _hooks
    antenv.axon_hooks = _hooks
    try:
        from trn_agent_boot.trn_boot import _ntff_profile_via_ctypes

        _hooks.set_axon_ntff_profile_hook(
            _ntff_profile_via_ctypes("/opt/axon/libaxon_pjrt.so")
        )
    except Exception:
        pass

import jax  # noqa: E402
import jax.numpy as jnp  # noqa: E402
import ml_dtypes  # noqa: E402
from jax.sharding import Mesh, NamedSharding, PartitionSpec as P  # noqa: E402
from jax.experimental.shard_map import shard_map  # noqa: E402

DIM = 768
DEPTH = 8
HEADS = 12
DH = 64
DFF = 2048
SOFTCLAMP = 50.0
B, T, S = 2, 32, 256
EPS = 1e-6
NC = 8
TL = T // NC  # 4 frames/core (t-domain)
SL = S // NC  # 32 positions/core (s-domain)
NTOK = B * TL * S  # 2048 tokens per core in either domain

DEBUG_DUMPS = os.environ.get("KV2_DEBUG", "0") == "1"


def _round_f32r(x):
    """fp32 -> fp32r (13 explicit mantissa bits, RNE) rounding on host."""
    u = np.ascontiguousarray(x, dtype=np.float32).view(np.uint32)
    lsb = (u >> 10) & 1
    r = (u + 0x1FF + lsb) & np.uint32(0xFFFFFC00)
    return r.view(np.float32).copy()


def _make_rotary(n):
    inv = 1.0 / (10000.0 ** (np.arange(0, DH, 2, dtype=np.float32) / DH))
    f = np.arange(n, dtype=np.float32)[:, None] * inv[None, :]
    return np.concatenate([f, f], axis=-1)  # (n, DH)


# ---------------------------------------------------------------------------
# host-side weight packing
# ---------------------------------------------------------------------------


def _pack_weights(inputs):
    """All-layer stacked, norm-folded weights (np arrays)."""
    f32 = np.float32
    bf16 = ml_dtypes.bfloat16
    anw = np.asarray(inputs["attn_norm_w"], f32)[:, :, None]  # (8, 768, 1)
    fnw = np.asarray(inputs["ff_norm_w"], f32)[:, :, None]
    g = {}
    g["Wq8"] = (np.asarray(inputs["Wq"], f32) * anw).astype(bf16)
    g["Wk8"] = (np.asarray(inputs["Wk"], f32) * anw).astype(bf16)
    g["Wv8"] = (np.asarray(inputs["Wv"], f32) * anw).astype(bf16)
    g["Wo8"] = np.asarray(inputs["Wo"], f32).astype(bf16)
    g["Wmg8"] = (
        np.concatenate(
            [
                np.asarray(inputs["Wmix"], f32) * anw,
                np.asarray(inputs["Wg"], f32) * anw,
            ],
            axis=2,
        )
    ).astype(bf16)  # (8, 768, 24)
    # k scale applied after l2norm; folds sqrt(DH), 1/sqrt(DH) and 1/softclamp
    g["kg8"] = (
        ((np.asarray(inputs["k_gamma"], f32) + 1.0) / SOFTCLAMP)
        .reshape(DEPTH, HEADS * DH)
        .astype(f32)
    )
    g["Win8"] = (np.asarray(inputs["Win"], f32) * fnw).astype(bf16)
    g["Wout8"] = np.asarray(inputs["Wout"], f32).astype(bf16)
    g["vrW"] = (
        np.asarray(inputs["vr_norm_w"], f32)[:, None]
        * np.asarray(inputs["vr_W"], f32)
    ).astype(bf16)
    # rotary tables for the time layers: partition p = seq*32 + t, t = p % 32
    rot = _make_rotary(T)  # (32, 64), halves identical
    tt = np.tile(np.arange(T), 4)  # (128,) t per partition
    g["rotc"] = np.cos(rot[tt, :]).astype(f32)  # (128, 64)
    g["rotsp"] = np.sin(rot[tt, :32]).astype(f32)  # (128, 32)
    g["rotsn"] = (-np.sin(rot[tt, :32])).astype(f32)  # (128, 32)
    # block-diag causal mask in [kv, q] layout: pk=sq*32+tk, pq=sq'*32+tq
    pk = np.arange(128)
    mask = (pk[:, None] // 32 == pk[None, :] // 32) & (
        pk[:, None] % 32 <= pk[None, :] % 32
    )
    g["maskt"] = mask.astype(bf16)  # (128, 128)
    return g


# ---------------------------------------------------------------------------
# Bass kernel
# ---------------------------------------------------------------------------
from contextlib import ExitStack  # noqa: E402

import concourse.bacc as bacc  # noqa: E402
import concourse.mybir as mybir  # noqa: E402
import concourse.tile as tile  # noqa: E402
from concourse.bass import ds  # noqa: E402
from concourse.masks import make_identity  # noqa: E402

F32 = mybir.dt.float32
F32R = mybir.dt.float32r
BF16 = mybir.dt.bfloat16
I32 = mybir.dt.int32
AF = mybir.ActivationFunctionType
OP = mybir.AluOpType

NT = 16  # token tiles (2048 tokens)
NSEQ = 8  # groups of 256 tokens
KT = 6  # 768 / 128 feature tiles
H = 12

ident_g = {}  # "r": f32r identity, "b": bf16 identity


def _emit_rsqrt(nc, pool, out, in_, scale, bias, guard):
    """out = 1/sqrt(max(in_*scale + bias, guard)); quake seed + 3 Newton."""
    shp = [128, in_.shape[1]]
    m = pool.tile(shp, F32, name="rs_m", tag="rs_m")
    nc.vector.tensor_scalar(m[:], in_, scale, bias, op0=OP.mult, op1=OP.add)
    nc.vector.tensor_scalar_max(m[:], m[:], guard)
    yi = pool.tile(shp, I32, name="rs_yi", tag="rs_yi")
    nc.vector.tensor_scalar(
        yi[:], m[:].bitcast(I32), 1, None, op0=OP.arith_shift_right
    )
    nc.vector.tensor_scalar(
        yi[:], yi[:], -1, 0x5F3759DF, op0=OP.mult, op1=OP.add
    )
    y = yi[:].bitcast(F32)
    half = pool.tile(shp, F32, name="rs_half", tag="rs_half")
    nc.vector.tensor_scalar_mul(half[:], m[:], 0.5)
    t1 = pool.tile(shp, F32, name="rs_t1", tag="rs_t1")
    for it in range(3):
        nc.vector.tensor_tensor(t1[:], y, y, op=OP.mult)
        nc.vector.tensor_tensor(t1[:], t1[:], half[:], op=OP.mult)
        nc.vector.tensor_scalar(t1[:], t1[:], -1.0, 1.5, op0=OP.mult, op1=OP.add)
        if it < 2:
            nc.vector.tensor_tensor(y, y, t1[:], op=OP.mult)
        else:
            nc.vector.tensor_tensor(out, y, t1[:], op=OP.mult)
    return out


def build_full():
    nc = bacc.Bacc(None, target_bir_lowering=False, num_devices=8)

    x_in = nc.dram_tensor("x_in", [NTOK, DIM], F32, kind="ExternalInput")
    Wq8 = nc.dram_tensor("Wq8", [DEPTH, 768, 768], BF16, kind="ExternalInput")
    Wk8 = nc.dram_tensor("Wk8", [DEPTH, 768, 768], BF16, kind="ExternalInput")
    Wv8 = nc.dram_tensor("Wv8", [DEPTH, 768, 768], BF16, kind="ExternalInput")
    Wo8 = nc.dram_tensor("Wo8", [DEPTH, 768, 768], BF16, kind="ExternalInput")
    Wmg8 = nc.dram_tensor("Wmg8", [DEPTH, 768, 24], BF16, kind="ExternalInput")
    kg8 = nc.dram_tensor("kg8", [DEPTH, 768], F32, kind="ExternalInput")
    Win8 = nc.dram_tensor("Win8", [DEPTH, 768, 4096], BF16, kind="ExternalInput")
    Wout8 = nc.dram_tensor("Wout8", [DEPTH, 2048, 768], BF16, kind="ExternalInput")
    vrW = nc.dram_tensor("vrW", [768, 768], BF16, kind="ExternalInput")
    rotc = nc.dram_tensor("rotc", [128, 64], F32, kind="ExternalInput")
    rotsp = nc.dram_tensor("rotsp", [128, 32], F32, kind="ExternalInput")
    rotsn = nc.dram_tensor("rotsn", [128, 32], F32, kind="ExternalInput")
    maskt = nc.dram_tensor("maskt", [128, 128], BF16, kind="ExternalInput")
    x_out = nc.dram_tensor("x_out", [NTOK, DIM], F32, kind="ExternalOutput")
    dbg = {}
    if DEBUG_DUMPS:
        for nm in ("dbg_rv", "dbg_x0", "dbg_x2", "dbg_x3", "dbg_x6"):
            dbg[nm] = nc.dram_tensor(nm, [NTOK, DIM], F32, kind="ExternalOutput")

    with tile.TileContext(nc) as tc:
        with ExitStack() as top:
            dram = top.enter_context(tc.tile_pool(name="dram", bufs=1, space="DRAM"))
            # bounce buffers: [j, (x|rv), b, tl, sl, d] for t->s at layer 3;
            # [j, b, sl, tl, d] for s->t; [j, b, tl, sl, d] for t->s at L7.
            b1_in = dram.tile([NC, 2, B, TL, SL, DIM], F32, name="b1_in")
            b1_out = dram.tile([NC, 2, B, TL, SL, DIM], F32, name="b1_out")
            b2_in = dram.tile([NC, B, SL, TL, DIM], F32, name="b2_in")
            b2_out = dram.tile([NC, B, SL, TL, DIM], F32, name="b2_out")
            b3_in = dram.tile([NC, B, TL, SL, DIM], F32, name="b3_in")
            b3_out = dram.tile([NC, B, TL, SL, DIM], F32, name="b3_out")

            const = top.enter_context(tc.tile_pool(name="const", bufs=1))
            xpool = top.enter_context(tc.tile_pool(name="xpool", bufs=1))
            x_sb = xpool.tile([128, NT, 768], F32, name="x_sb")
            nc.sync.dma_start(
                x_sb[:], x_in[:].rearrange("(t p) d -> p t d", p=128)
            )
            ident_f = const.tile([128, 128], F32, name="ident_f")
            make_identity(nc, ident_f)
            ident_r = const.tile([128, 128], F32R, name="ident_r")
            nc.vector.tensor_copy(ident_r[:], ident_f[:])
            ident_b = const.tile([128, 128], BF16, name="ident_b")
            nc.vector.tensor_copy(ident_b[:], ident_f[:])
            ident_g["r"] = ident_r
            ident_g["b"] = ident_b
            rc_sb = const.tile([128, 64], F32, name="rc_sb")
            rsp_sb = const.tile([128, 32], F32, name="rsp_sb")
            rsn_sb = const.tile([128, 32], F32, name="rsn_sb")
            mask_sb = const.tile([128, 128], BF16, name="mask_sb")
            nc.sync.dma_start(rc_sb[:], rotc[:])
            nc.sync.dma_start(rsp_sb[:], rotsp[:])
            nc.sync.dma_start(rsn_sb[:], rotsn[:])
            nc.sync.dma_start(mask_sb[:], maskt[:])

            # ---- pre: rv = rmsnorm(tok) @ vrW -> b1_in[:, 1] blocks ----
            _pre_rv(nc, tc, x_sb, vrW, b1_in)
            if DEBUG_DUMPS:
                _dump_rv(nc, b1_in, dbg["dbg_rv"])

            for L in (0, 1, 2):
                _attn_space(nc, tc, L, x_sb, b1_in, Wq8, Wk8, Wv8, Wo8, Wmg8, kg8)
                _ff_layer(nc, tc, L, x_sb, Win8, Wout8)
                if DEBUG_DUMPS and L == 0:
                    nc.sync.dma_start(
                        dbg["dbg_x0"][:].rearrange("(t p) d -> p t d", p=128),
                        x_sb[:],
                    )
            if DEBUG_DUMPS:
                nc.sync.dma_start(
                    dbg["dbg_x2"][:].rearrange("(t p) d -> p t d", p=128), x_sb[:]
                )

            # ---- reshard t->s (x into b1_in[:,0]), AllToAll, load ----
            _t2s_out(nc, x_sb, b1_in, xr=0)
            nc.gpsimd.collective_compute(
                "AllToAll", OP.bypass, replica_groups=[list(range(NC))],
                ins=[b1_in.opt()], outs=[b1_out.opt()],
            )
            _load_s(nc, x_sb, b1_out, xr=0)

            _attn_time(nc, tc, 3, x_sb, b1_out, Wq8, Wk8, Wv8, Wo8, Wmg8, kg8,
                       rc_sb, rsp_sb, rsn_sb, mask_sb)
            _ff_layer(nc, tc, 3, x_sb, Win8, Wout8)
            if DEBUG_DUMPS:
                nc.sync.dma_start(
                    dbg["dbg_x3"][:].rearrange("(t p) d -> p t d", p=128), x_sb[:]
                )

            # ---- reshard s->t, AllToAll, load ----
            _s2t_out(nc, x_sb, b2_in)
            nc.gpsimd.collective_compute(
                "AllToAll", OP.bypass, replica_groups=[list(range(NC))],
                ins=[b2_in.opt()], outs=[b2_out.opt()],
            )
            _load_t(nc, x_sb, b2_out)

            for L in (4, 5, 6):
                _attn_space(nc, tc, L, x_sb, b1_in, Wq8, Wk8, Wv8, Wo8, Wmg8, kg8)
                _ff_layer(nc, tc, L, x_sb, Win8, Wout8)
            if DEBUG_DUMPS:
                nc.sync.dma_start(
                    dbg["dbg_x6"][:].rearrange("(t p) d -> p t d", p=128), x_sb[:]
                )

            # ---- reshard t->s for layer 7 ----
            _t2s_out(nc, x_sb, b3_in, xr=None)
            nc.gpsimd.collective_compute(
                "AllToAll", OP.bypass, replica_groups=[list(range(NC))],
                ins=[b3_in.opt()], outs=[b3_out.opt()],
            )
            _load_s(nc, x_sb, b3_out, xr=None)

            _attn_time(nc, tc, 7, x_sb, b1_out, Wq8, Wk8, Wv8, Wo8, Wmg8, kg8,
                       rc_sb, rsp_sb, rsn_sb, mask_sb)
            _ff_layer(nc, tc, 7, x_sb, Win8, Wout8)

            _final_norm(nc, tc, x_sb, x_out)

    nc.compile()
    return nc


# ---------------------------------------------------------------------------
# reshard DMA helpers (all partition APs are contiguous ranges)
# ---------------------------------------------------------------------------


def _t2s_out(nc, x_sb, bounce, xr):
    """x_sb (t-domain) -> bounce blocks [j, (xr,) b, tl, sl, d]."""
    for j in range(NC):
        src = (
            x_sb[ds(32 * (j % 4), 32), :, :]
            .rearrange("p (bt two) d -> p bt two d", two=2)[:, :, j // 4, :]
        )  # [32(sl), 8(bt), 768]
        blk = bounce[j, xr] if xr is not None else bounce[j]
        dst = blk.rearrange("b tl sl d -> sl b tl d")  # [32, 2, 4, 768]
        nc.sync.dma_start(dst, src)


def _load_s(nc, x_sb, bounce, xr):
    """bounce (post-a2a, [c, (xr,) b, tl, sl, d]) -> x_sb s-domain tiles."""
    for g in range(NT):
        b, sl0 = g // 8, (g % 8) * 4
        blk = bounce[:, xr] if xr is not None else bounce[:]
        for sq in range(4):
            src = blk[:, b, :, sl0 + sq, :]  # [8(c), 4(tl), 768]
            nc.sync.dma_start(x_sb[ds(32 * sq, 32), g, :], src)


def _s2t_out(nc, x_sb, bounce):
    """x_sb (s-domain) -> bounce blocks [j, b, sl, tl, d]."""
    for j in range(NC):
        for sq in range(4):
            src = x_sb[ds(32 * sq + 4 * j, 4), :, :]  # [4(tl), 16(g), 768]
            dst = bounce[j].rearrange(
                "b (gg sq) tl d -> sq tl b gg d", sq=4
            )[sq]  # [4(tl), 2(b), 8(gg), 768]
            nc.sync.dma_start(dst, src)


def _load_t(nc, x_sb, bounce):
    """bounce (post-a2a, [c, b, sl, tl, d]) -> x_sb t-domain tiles."""
    for tt in range(NT):
        bt, half = tt // 2, tt % 2
        b, tl = bt // 4, bt % 4
        src = bounce[ds(4 * half, 4), b, :, tl, :]  # [4(c), 32(sl), 768]
        nc.sync.dma_start(x_sb[:, tt, :], src)


def _dump_rv(nc, b1_in, dst):
    """debug: rv blocks -> t-domain rows [2048, 768]."""
    # b1_in[:, 1] = [j, b, tl, sl, d]; row (b, tl, s=j*32+sl)
    for b in range(B):
        for tl in range(TL):
            src = b1_in[:, 1, b, tl]  # [j, sl, d]
            dstv = dst[ds((b * TL + tl) * S, S), :]  # [256, 768]
            nc.sync.dma_start(dstv, src)


# ---------------------------------------------------------------------------
# model phases
# ---------------------------------------------------------------------------


def _rmsnorm_tiles(nc, sp, np_, x_sb, off, nj, pfx, dt=BF16):
    """rmsnorm of nj consecutive token tiles -> tile [128, nj, 768] (dt)."""
    sq = sp.tile([128, 768], F32, name=f"{pfx}sq", tag=f"{pfx}sq")
    ss = np_.tile([128, nj], F32, name=f"{pfx}ss", tag=f"{pfx}ss")
    for j in range(nj):
        nc.scalar.activation(
            sq[:], x_sb[:, ds(off + j, 1), :].squeeze(1), AF.Square,
            accum_out=ss[:, j : j + 1],
        )
    inv = np_.tile([128, nj], F32, name=f"{pfx}inv", tag=f"{pfx}inv")
    _emit_rsqrt(nc, np_, inv[:], ss[:], 1.0 / 768.0, 1e-6, 1e-30)
    tn_t = sp.tile([128, nj, 768], dt, name=f"{pfx}tn", tag=f"{pfx}tn")
    for j in range(nj):
        nc.vector.tensor_scalar_mul(
            tn_t[:, j, :], x_sb[:, ds(off + j, 1), :].squeeze(1),
            inv[:, j : j + 1],
        )
    return sq, tn_t


def _transpose_bf(nc, ps_tr, src_t, dst, nj):
    """token-major [128, nj, 768] bf16 -> feature-major [128, KT, nj*128]."""
    for kt in range(KT):
        pt = ps_tr.tile([128, nj * 128], BF16, name="pt", tag="ps_trb")
        for j in range(nj):
            nc.tensor.transpose(
                pt[:, j * 128 : (j + 1) * 128],
                src_t[:, j, kt * 128 : (kt + 1) * 128],
                ident_g["b"][:],
            )
        nc.scalar.copy(dst[:, kt, :], pt[:])


def _pre_rv(nc, tc, x_sb, vrW, b1_in):
    with ExitStack() as ctx:
        wp = ctx.enter_context(tc.tile_pool(name="wvr", bufs=1))
        wvr = wp.tile([128, KT, 768], BF16, name="wvr_t")
        nc.sync.dma_start(wvr[:], vrW[:].rearrange("(kt p) m -> p kt m", p=128))
        sp = ctx.enter_context(tc.tile_pool(name="prsp", bufs=2))
        np_ = ctx.enter_context(tc.tile_pool(name="prnp", bufs=2))
        ps_tr = ctx.enter_context(
            tc.tile_pool(name="prps_tr", bufs=2, space="PSUM")
        )
        ps_pj = ctx.enter_context(
            tc.tile_pool(name="prps_pj", bufs=2, space="PSUM")
        )
        for sv in range(NSEQ):
            b, tl = sv // 4, sv % 4
            _, tn_t = _rmsnorm_tiles(nc, sp, np_, x_sb, sv * 2, 2, "pr")
            tn_f = sp.tile([128, KT, 256], BF16, name="prtn_f", tag="prtn_f")
            _transpose_bf(nc, ps_tr, tn_t, tn_f, 2)
            rv_t = sp.tile([128, 2, 768], F32, name="rv_t", tag="rv_t")
            for j in range(2):
                for nh in range(2):
                    pv = ps_pj.tile([128, 384], F32, name="pv", tag="ps_pj")
                    for kt in range(KT):
                        nc.tensor.matmul(
                            pv[:],
                            lhsT=tn_f[:, kt, j * 128 : (j + 1) * 128],
                            rhs=wvr[:, kt, nh * 384 : (nh + 1) * 384],
                            start=(kt == 0),
                            stop=(kt == KT - 1),
                        )
                    nc.scalar.copy(rv_t[:, j, nh * 384 : (nh + 1) * 384], pv[:])
            for jj in range(2):
                nc.sync.dma_start(
                    b1_in[ds(4 * jj, 4), 1, b, tl, :, :], rv_t[:, jj, :]
                )


def _attn_space(nc, tc, L, x_sb, rv_blocks, Wq8, Wk8, Wv8, Wo8, Wmg8, kg8):
    with ExitStack() as ctx:
        wp = ctx.enter_context(tc.tile_pool(name=f"wq{L}", bufs=1))
        wq = wp.tile([128, KT, 768], BF16, name=f"wq_t{L}")
        wk = wp.tile([128, KT, 768], BF16, name=f"wk_t{L}")
        wv = wp.tile([128, KT, 768], BF16, name=f"wv_t{L}")
        wo = wp.tile([128, KT, 768], BF16, name=f"wo_t{L}")
        wmg = wp.tile([128, KT, 24], BF16, name=f"wmg_t{L}")
        kgbc = wp.tile([128, 768], F32, name=f"kgbc{L}")
        for w_t, W in ((wq, Wq8), (wk, Wk8), (wv, Wv8), (wo, Wo8), (wmg, Wmg8)):
            nc.sync.dma_start(
                w_t[:], W[L].rearrange("(kt p) m -> p kt m", p=128)
            )
        nc.sync.dma_start(kgbc[:], kg8[L : L + 1, :].partition_broadcast(128))

        sp = ctx.enter_context(tc.tile_pool(name=f"sp{L}", bufs=2))
        sp2 = ctx.enter_context(tc.tile_pool(name=f"sp2{L}", bufs=2))
        hp = ctx.enter_context(tc.tile_pool(name=f"hp{L}", bufs=3))
        np_ = ctx.enter_context(tc.tile_pool(name=f"np{L}", bufs=2))
        ps_trb = ctx.enter_context(
            tc.tile_pool(name=f"ps_trb{L}", bufs=2, space="PSUM")
        )
        ps_pj = ctx.enter_context(
            tc.tile_pool(name=f"ps_pj{L}", bufs=2, space="PSUM")
        )
        ps_S = ctx.enter_context(
            tc.tile_pool(name=f"ps_S{L}", bufs=2, space="PSUM")
        )
        ps_O = ctx.enter_context(
            tc.tile_pool(name=f"ps_O{L}", bufs=2, space="PSUM")
        )

        def seq_body(sv):
            off = sv * 2
            b, tl = sv // 4, sv % 4
            # ---- rv slice for this seq (from b1_in xr=1 blocks)
            rv_sl = sp.tile([128, 2, 768], F32, name="rv_sl", tag="rv_sl")
            for jj in range(2):
                nc.sync.dma_start(
                    rv_sl[:, jj, :], rv_blocks[ds(4 * jj, 4), 1, b, tl, :, :]
                )
            # ---- rmsnorm -> bf16 tn
            sq, tn_t = _rmsnorm_tiles(nc, sp, np_, x_sb, off, 2, "a")
            tn_f = sp.tile([128, KT, 256], BF16, name="tn_f", tag="tn_f")
            _transpose_bf(nc, ps_trb, tn_t, tn_f, 2)
            # ---- q projection (feature-major)
            q_f = sp2.tile([128, KT, 256], BF16, name="q_f", tag="q_f")
            for m in range(KT):
                pq = ps_pj.tile([128, 384], F32, name="pq", tag="ps_pj")
                for kt in range(KT):
                    nc.tensor.matmul(
                        pq[:, :256],
                        lhsT=wq[:, kt, m * 128 : (m + 1) * 128],
                        rhs=tn_f[:, kt, :],
                        start=(kt == 0),
                        stop=(kt == KT - 1),
                    )
                nc.scalar.copy(q_f[:, m, :], pq[:, :256])
            # ---- k projection (token-major) + l2norm * kgamma
            kraw = sp.tile([128, 2, 768], BF16, name="kraw", tag="kraw")
            for j in range(2):
                for nh in range(2):
                    pk = ps_pj.tile([128, 384], F32, name="pk", tag="ps_pj")
                    for kt in range(KT):
                        nc.tensor.matmul(
                            pk[:],
                            lhsT=tn_f[:, kt, j * 128 : (j + 1) * 128],
                            rhs=wk[:, kt, nh * 384 : (nh + 1) * 384],
                            start=(kt == 0),
                            stop=(kt == KT - 1),
                        )
                    nc.scalar.copy(kraw[:, j, nh * 384 : (nh + 1) * 384], pk[:])
            kss = np_.tile([128, 24], F32, name="kss", tag="kss")
            for j in range(2):
                nc.vector.tensor_tensor(
                    sq[:], kraw[:, j, :], kraw[:, j, :], op=OP.mult
                )
                nc.vector.tensor_reduce(
                    out=kss[:, j * 12 : (j + 1) * 12],
                    in_=sq[:].rearrange("p (h d) -> p h d", h=H),
                    axis=mybir.AxisListType.X,
                    op=OP.add,
                )
            kinv = np_.tile([128, 24], F32, name="kinv", tag="kinv")
            _emit_rsqrt(nc, np_, kinv[:], kss[:], 1.0, 0.0, 1e-24)
            kib = sp.tile([128, 768], F32, name="kib", tag="kib")
            for j in range(2):
                nc.vector.tensor_copy(
                    kib[:].rearrange("p (h d) -> p h d", h=H),
                    kinv[:, j * 12 : (j + 1) * 12]
                    .unsqueeze(2)
                    .broadcast_to([128, H, DH]),
                )
                nc.vector.tensor_tensor(kib[:], kib[:], kgbc[:], op=OP.mult)
                nc.vector.tensor_tensor(
                    kraw[:, j, :], kraw[:, j, :], kib[:], op=OP.mult
                )
            k_f = sp2.tile([128, KT, 256], BF16, name="k_f", tag="k_f")
            _transpose_bf(nc, ps_trb, kraw, k_f, 2)
            # ---- mix / gates (sigmoid via tanh)
            mgs = np_.tile([128, 2, 24], F32, name="mgs", tag="mgs")
            for j in range(2):
                pm = ps_O.tile([128, 65], F32, name="pm", tag="ps_O")
                for kt in range(KT):
                    nc.tensor.matmul(
                        pm[:, :24],
                        lhsT=tn_f[:, kt, j * 128 : (j + 1) * 128],
                        rhs=wmg[:, kt, :],
                        start=(kt == 0),
                        stop=(kt == KT - 1),
                    )
                nc.scalar.activation(mgs[:, j, :], pm[:, :24], AF.Tanh, scale=0.5)
            nc.vector.tensor_scalar(
                mgs[:], mgs[:], 0.5, 0.5, op0=OP.mult, op1=OP.add
            )
            # ---- v projection + value-residual lerp -> v1 (bf16, |1 col)
            v1 = sp2.tile([128, 2, H, 65], BF16, name="v1", tag="v1")
            mixb = kib
            tdt = sq[:, 0:384]
            for j in range(2):
                nc.vector.tensor_copy(
                    mixb[:].rearrange("p (h d) -> p h d", h=H),
                    mgs[:, j, 0:12].unsqueeze(2).broadcast_to([128, H, DH]),
                )
                for nh in range(2):
                    pv = ps_pj.tile([128, 384], F32, name="pv", tag="ps_pj")
                    for kt in range(KT):
                        nc.tensor.matmul(
                            pv[:],
                            lhsT=tn_f[:, kt, j * 128 : (j + 1) * 128],
                            rhs=wv[:, kt, nh * 384 : (nh + 1) * 384],
                            start=(kt == 0),
                            stop=(kt == KT - 1),
                        )
                    nc.vector.tensor_tensor(
                        tdt, rv_sl[:, j, nh * 384 : (nh + 1) * 384], pv[:],
                        op=OP.subtract,
                    )
                    nc.vector.tensor_tensor(
                        tdt, tdt, mixb[:, nh * 384 : (nh + 1) * 384],
                        op=OP.mult,
                    )
                    nc.vector.tensor_tensor(
                        v1[:, j, 6 * nh : 6 * nh + 6, 0:64],
                        pv[:].rearrange("p (h d) -> p h d", h=6),
                        tdt.rearrange("p (h d) -> p h d", h=6),
                        op=OP.add,
                    )
                nc.vector.memset(v1[:, j, :, 64:65], 1.0)
            # ---- attention per head
            o_t = sp.tile([128, 2, 768], BF16, name="o_t", tag="o_t")
            for h in range(H):
                s_t = hp.tile([128, 2, 256], BF16, name="s_t", tag="s_t")
                pt_b = hp.tile([128, 2, 256], BF16, name="pt_b", tag="pt_b")
                rec = np_.tile([128, 1], F32, name="rec", tag="rec")
                mt, po = h // 2, 64 * (h % 2)
                for qt in range(2):
                    pS = ps_S.tile([128, 256], F32, name="pS", tag="ps_S")
                    nc.tensor.matmul(
                        pS[:],
                        lhsT=q_f[po : po + 64, mt, qt * 128 : (qt + 1) * 128],
                        rhs=k_f[po : po + 64, mt, :],
                        start=True,
                        stop=True,
                    )
                    nc.scalar.activation(s_t[:, qt, :], pS[:], AF.Tanh)
                for kvt in range(2):
                    ppt = ps_trb.tile([128, 256], BF16, name="ppt", tag="ps_trb")
                    for qt in range(2):
                        nc.tensor.transpose(
                            ppt[:, qt * 128 : (qt + 1) * 128],
                            s_t[:, qt, kvt * 128 : (kvt + 1) * 128],
                            ident_g["b"][:],
                        )
                    nc.scalar.activation(
                        pt_b[:, kvt, :], ppt[:], AF.Exp, scale=50.0
                    )
                for qt in range(2):
                    pO = ps_O.tile([128, 65], F32, name="pO", tag="ps_O")
                    for kvt in range(2):
                        nc.tensor.matmul(
                            pO[:],
                            lhsT=pt_b[:, kvt, qt * 128 : (qt + 1) * 128],
                            rhs=v1[:, kvt, h, :],
                            start=(kvt == 0),
                            stop=(kvt == 1),
                        )
                    nc.vector.reciprocal(rec[:], pO[:, 64:65])
                    nc.vector.tensor_tensor(
                        rec[:], rec[:], mgs[:, qt, 12 + h : 13 + h], op=OP.mult
                    )
                    nc.vector.tensor_scalar_mul(
                        o_t[:, qt, 64 * h : 64 * h + 64], pO[:, 0:64], rec[:]
                    )
            # ---- transpose o -> o_f, then Wo and residual add
            o_f = sp.tile([128, KT, 256], BF16, name="o_f", tag="o_f")
            _transpose_bf(nc, ps_trb, o_t, o_f, 2)
            for j in range(2):
                for nh in range(2):
                    px = ps_pj.tile([128, 384], F32, name="px", tag="ps_pj")
                    for kt in range(KT):
                        nc.tensor.matmul(
                            px[:],
                            lhsT=o_f[:, kt, j * 128 : (j + 1) * 128],
                            rhs=wo[:, kt, nh * 384 : (nh + 1) * 384],
                            start=(kt == 0),
                            stop=(kt == KT - 1),
                        )
                    xs = x_sb[:, ds(off + j, 1), nh * 384 : (nh + 1) * 384]
                    xs = xs.squeeze(1)
                    nc.vector.tensor_tensor(xs, xs, px[:], op=OP.add)

        for _sv in range(NSEQ):
            seq_body(_sv)


def _attn_time(nc, tc, L, x_sb, b1_out, Wq8, Wk8, Wv8, Wo8, Wmg8, kg8,
               rc_sb, rsp_sb, rsn_sb, mask_sb):
    with ExitStack() as ctx:
        wp = ctx.enter_context(tc.tile_pool(name=f"twq{L}", bufs=1))
        wq = wp.tile([128, KT, 768], BF16, name=f"twq_t{L}")
        wk = wp.tile([128, KT, 768], BF16, name=f"twk_t{L}")
        wv = wp.tile([128, KT, 768], BF16, name=f"twv_t{L}")
        wo = wp.tile([128, KT, 768], BF16, name=f"two_t{L}")
        wmg = wp.tile([128, KT, 24], BF16, name=f"twmg_t{L}")
        kgbc = wp.tile([128, 768], F32, name=f"tkgbc{L}")
        for w_t, W in ((wq, Wq8), (wk, Wk8), (wv, Wv8), (wo, Wo8), (wmg, Wmg8)):
            nc.sync.dma_start(
                w_t[:], W[L].rearrange("(kt p) m -> p kt m", p=128)
            )
        nc.sync.dma_start(kgbc[:], kg8[L : L + 1, :].partition_broadcast(128))

        sp = ctx.enter_context(tc.tile_pool(name=f"tsp{L}", bufs=2))
        sp2 = ctx.enter_context(tc.tile_pool(name=f"tsp2{L}", bufs=2))
        hp = ctx.enter_context(tc.tile_pool(name=f"thp{L}", bufs=3))
        np_ = ctx.enter_context(tc.tile_pool(name=f"tnp{L}", bufs=2))
        ps_trb = ctx.enter_context(
            tc.tile_pool(name=f"tps_trb{L}", bufs=2, space="PSUM")
        )
        ps_pj = ctx.enter_context(
            tc.tile_pool(name=f"tps_pj{L}", bufs=2, space="PSUM")
        )
        ps_S = ctx.enter_context(
            tc.tile_pool(name=f"tps_S{L}", bufs=2, space="PSUM")
        )
        ps_O = ctx.enter_context(
            tc.tile_pool(name=f"tps_O{L}", bufs=2, space="PSUM")
        )

        rc_bc = rc_sb[:].unsqueeze(1).broadcast_to([128, H, 64])
        rsp_bc = rsp_sb[:].unsqueeze(1).broadcast_to([128, H, 32])
        rsn_bc = rsn_sb[:].unsqueeze(1).broadcast_to([128, H, 32])

        def _rotary(x_r, rtmp, rtmp2):
            """x_r [128, 768] bf16 view; rotate in place."""
            xv = x_r.rearrange("p (h half d) -> p h half d", h=H, half=2)
            tv = rtmp[:].rearrange("p (h half d) -> p h half d", h=H, half=2)
            nc.vector.tensor_tensor(tv[:, :, 0, :], xv[:, :, 1, :], rsn_bc,
                                    op=OP.mult)
            nc.vector.tensor_tensor(tv[:, :, 1, :], xv[:, :, 0, :], rsp_bc,
                                    op=OP.mult)
            nc.vector.tensor_tensor(
                rtmp2[:].rearrange("p (h d) -> p h d", h=H),
                x_r.rearrange("p (h d) -> p h d", h=H), rc_bc, op=OP.mult
            )
            nc.vector.tensor_tensor(x_r, rtmp2[:], rtmp[:], op=OP.add)

        def seq_body(sv):
            off = sv * 2
            b = sv // 4
            # ---- rv slice (s-domain blocks from b1_out xr=1)
            rv_sl = sp.tile([128, 2, 768], F32, name="trv_sl", tag="trv_sl")
            for jj in range(2):
                sl0 = ((off + jj) % 8) * 4
                for sq_i in range(4):
                    nc.sync.dma_start(
                        rv_sl[ds(32 * sq_i, 32), jj, :],
                        b1_out[:, 1, b, :, sl0 + sq_i, :],
                    )
            # ---- rmsnorm -> bf16 tn
            sq, tn_t = _rmsnorm_tiles(nc, sp, np_, x_sb, off, 2, "t")
            tn_f = sp.tile([128, KT, 256], BF16, name="ttn_f", tag="ttn_f")
            _transpose_bf(nc, ps_trb, tn_t, tn_f, 2)
            rtmp = sp.tile([128, 768], F32, name="rtmp", tag="rtmp")
            rtmp2 = sp.tile([128, 768], F32, name="rtmp2", tag="rtmp2")
            # ---- q projection token-major + rotary -> transpose to q_f
            qraw = sp.tile([128, 2, 768], BF16, name="qraw", tag="qraw")
            for j in range(2):
                for nh in range(2):
                    pq = ps_pj.tile([128, 384], F32, name="tpq", tag="ps_pj")
                    for kt in range(KT):
                        nc.tensor.matmul(
                            pq[:],
                            lhsT=tn_f[:, kt, j * 128 : (j + 1) * 128],
                            rhs=wq[:, kt, nh * 384 : (nh + 1) * 384],
                            start=(kt == 0),
                            stop=(kt == KT - 1),
                        )
                    nc.scalar.copy(qraw[:, j, nh * 384 : (nh + 1) * 384], pq[:])
                _rotary(qraw[:, j, :], rtmp, rtmp2)
            q_f = sp2.tile([128, KT, 256], BF16, name="tq_f", tag="tq_f")
            _transpose_bf(nc, ps_trb, qraw, q_f, 2)
            # ---- k projection + l2norm*kgamma + rotary -> k_f
            kraw = sp.tile([128, 2, 768], BF16, name="tkraw", tag="tkraw")
            for j in range(2):
                for nh in range(2):
                    pk = ps_pj.tile([128, 384], F32, name="tpk", tag="ps_pj")
                    for kt in range(KT):
                        nc.tensor.matmul(
                            pk[:],
                            lhsT=tn_f[:, kt, j * 128 : (j + 1) * 128],
                            rhs=wk[:, kt, nh * 384 : (nh + 1) * 384],
                            start=(kt == 0),
                            stop=(kt == KT - 1),
                        )
                    nc.scalar.copy(kraw[:, j, nh * 384 : (nh + 1) * 384], pk[:])
            kss = np_.tile([128, 24], F32, name="tkss", tag="tkss")
            for j in range(2):
                nc.vector.tensor_tensor(
                    sq[:], kraw[:, j, :], kraw[:, j, :], op=OP.mult
                )
                nc.vector.tensor_reduce(
                    out=kss[:, j * 12 : (j + 1) * 12],
                    in_=sq[:].rearrange("p (h d) -> p h d", h=H),
                    axis=mybir.AxisListType.X,
                    op=OP.add,
                )
            kinv = np_.tile([128, 24], F32, name="tkinv", tag="tkinv")
            _emit_rsqrt(nc, np_, kinv[:], kss[:], 1.0, 0.0, 1e-24)
            kib = sp.tile([128, 768], F32, name="tkib", tag="tkib")
            for j in range(2):
                nc.vector.tensor_copy(
                    kib[:].rearrange("p (h d) -> p h d", h=H),
                    kinv[:, j * 12 : (j + 1) * 12]
                    .unsqueeze(2)
                    .broadcast_to([128, H, DH]),
                )
                nc.vector.tensor_tensor(kib[:], kib[:], kgbc[:], op=OP.mult)
                nc.vector.tensor_tensor(
                    kraw[:, j, :], kraw[:, j, :], kib[:], op=OP.mult
                )
                _rotary(kraw[:, j, :], rtmp, rtmp2)
            k_f = sp2.tile([128, KT, 256], BF16, name="tk_f", tag="tk_f")
            _transpose_bf(nc, ps_trb, kraw, k_f, 2)
            # ---- mix / gates
            mgs = np_.tile([128, 2, 24], F32, name="tmgs", tag="tmgs")
            for j in range(2):
                pm = ps_O.tile([128, 65], F32, name="tpm", tag="ps_O")
                for kt in range(KT):
                    nc.tensor.matmul(
                        pm[:, :24],
                        lhsT=tn_f[:, kt, j * 128 : (j + 1) * 128],
                        rhs=wmg[:, kt, :],
                        start=(kt == 0),
                        stop=(kt == KT - 1),
                    )
                nc.scalar.activation(mgs[:, j, :], pm[:, :24], AF.Tanh, scale=0.5)
            nc.vector.tensor_scalar(
                mgs[:], mgs[:], 0.5, 0.5, op0=OP.mult, op1=OP.add
            )
            # ---- v projection + value-residual lerp -> v1
            v1 = sp2.tile([128, 2, H, 65], BF16, name="tv1", tag="tv1")
            mixb = kib
            tdt = sq[:, 0:384]
            for j in range(2):
                nc.vector.tensor_copy(
                    mixb[:].rearrange("p (h d) -> p h d", h=H),
                    mgs[:, j, 0:12].unsqueeze(2).broadcast_to([128, H, DH]),
                )
                for nh in range(2):
                    pv = ps_pj.tile([128, 384], F32, name="tpv", tag="ps_pj")
                    for kt in range(KT):
                        nc.tensor.matmul(
                            pv[:],
                            lhsT=tn_f[:, kt, j * 128 : (j + 1) * 128],
                            rhs=wv[:, kt, nh * 384 : (nh + 1) * 384],
                            start=(kt == 0),
                            stop=(kt == KT - 1),
                        )
                    nc.vector.tensor_tensor(
                        tdt, rv_sl[:, j, nh * 384 : (nh + 1) * 384], pv[:],
                        op=OP.subtract,
                    )
                    nc.vector.tensor_tensor(
                        tdt, tdt, mixb[:, nh * 384 : (nh + 1) * 384],
                        op=OP.mult,
                    )
                    nc.vector.tensor_tensor(
                        v1[:, j, 6 * nh : 6 * nh + 6, 0:64],
                        pv[:].rearrange("p (h d) -> p h d", h=6),
                        tdt.rearrange("p (h d) -> p h d", h=6),
                        op=OP.add,
                    )
                nc.vector.memset(v1[:, j, :, 64:65], 1.0)
            # ---- attention per head (block-local: kv tile == q tile)
            o_t = sp.tile([128, 2, 768], BF16, name="to_t", tag="to_t")
            for h in range(H):
                rec = np_.tile([128, 1], F32, name="trec", tag="trec")
                mt, po = h // 2, 64 * (h % 2)
                for tt in range(2):
                    pS = ps_S.tile([128, 256], F32, name="tpS", tag="ps_S")
                    nc.tensor.matmul(
                        pS[:, :128],
                        lhsT=q_f[po : po + 64, mt, tt * 128 : (tt + 1) * 128],
                        rhs=k_f[po : po + 64, mt, tt * 128 : (tt + 1) * 128],
                        start=True,
                        stop=True,
                    )
                    s_t = hp.tile([128, 128], BF16, name="ts_t", tag="ts_t")
                    nc.scalar.activation(s_t[:], pS[:, :128], AF.Tanh)
                    ppt = ps_trb.tile([128, 256], BF16, name="tppt", tag="ps_trb")
                    nc.tensor.transpose(ppt[:, :128], s_t[:], ident_g["b"][:])
                    pt_b = hp.tile([128, 128], BF16, name="tpt_b", tag="tpt_b")
                    nc.scalar.activation(
                        pt_b[:], ppt[:, :128], AF.Exp, scale=50.0
                    )
                    nc.vector.tensor_tensor(
                        pt_b[:], pt_b[:], mask_sb[:], op=OP.mult
                    )
                    pO = ps_O.tile([128, 65], F32, name="tpO", tag="ps_O")
                    nc.tensor.matmul(
                        pO[:], lhsT=pt_b[:], rhs=v1[:, tt, h, :],
                        start=True, stop=True,
                    )
                    nc.vector.reciprocal(rec[:], pO[:, 64:65])
                    nc.vector.tensor_tensor(
                        rec[:], rec[:], mgs[:, tt, 12 + h : 13 + h], op=OP.mult
                    )
                    nc.vector.tensor_scalar_mul(
                        o_t[:, tt, 64 * h : 64 * h + 64], pO[:, 0:64], rec[:]
                    )
            # ---- transpose o -> o_f, then Wo and residual add
            o_f = sp.tile([128, KT, 256], BF16, name="to_f", tag="to_f")
            _transpose_bf(nc, ps_trb, o_t, o_f, 2)
            for j in range(2):
                for nh in range(2):
                    px = ps_pj.tile([128, 384], F32, name="tpx", tag="ps_pj")
                    for kt in range(KT):
                        nc.tensor.matmul(
                            px[:],
                            lhsT=o_f[:, kt, j * 128 : (j + 1) * 128],
                            rhs=wo[:, kt, nh * 384 : (nh + 1) * 384],
                            start=(kt == 0),
                            stop=(kt == KT - 1),
                        )
                    xs = x_sb[:, ds(off + j, 1), nh * 384 : (nh + 1) * 384]
                    xs = xs.squeeze(1)
                    nc.vector.tensor_tensor(xs, xs, px[:], op=OP.add)

        for _sv in range(NSEQ):
            seq_body(_sv)


def _ff_layer(nc, tc, L, x_sb, Win8, Wout8):
    with ExitStack() as ctx:
        wop = ctx.enter_context(tc.tile_pool(name=f"wop{L}", bufs=1))
        win = wop.tile([128, KT, 4096], BF16, name=f"win_t{L}")
        nc.sync.dma_start(
            win[:], Win8[L].rearrange("(kt p) m -> p kt m", p=128)
        )
        wout = wop.tile([128, 16, 768], BF16, name=f"wout_t{L}")
        nc.sync.dma_start(
            wout[:], Wout8[L].rearrange("(kt p) m -> p kt m", p=128)
        )
        sp = ctx.enter_context(tc.tile_pool(name=f"fsp{L}", bufs=2))
        up = ctx.enter_context(tc.tile_pool(name=f"fup{L}", bufs=2))
        np_ = ctx.enter_context(tc.tile_pool(name=f"fnp{L}", bufs=2))
        ps_tr = ctx.enter_context(
            tc.tile_pool(name=f"fps_tr{L}", bufs=2, space="PSUM")
        )
        ps_h = ctx.enter_context(
            tc.tile_pool(name=f"fps_h{L}", bufs=4, space="PSUM")
        )
        ps_xd = ctx.enter_context(
            tc.tile_pool(name=f"fps_xd{L}", bufs=2, space="PSUM")
        )

        def chunk_body(cv):
            coff = cv * 4
            _, tn2 = _rmsnorm_tiles(nc, sp, np_, x_sb, coff, 4, "f")
            tn2f = sp.tile([128, KT, 512], BF16, name="tn2f", tag="tn2f")
            for kt in range(KT):
                pt = ps_tr.tile([128, 512], BF16, name="fpt", tag="fps_tr")
                for j in range(4):
                    nc.tensor.transpose(
                        pt[:, j * 128 : (j + 1) * 128],
                        tn2[:, j, kt * 128 : (kt + 1) * 128],
                        ident_g["b"][:],
                    )
                nc.scalar.copy(tn2f[:, kt, :], pt[:])
            # ---- h = tn2 @ Win; u = a * gelu(g)
            u = up.tile([128, 16, 512], BF16, name="u", tag="u")
            gl = sp.tile([128, 512], F32, name="gl", tag="gl")
            for m in range(16):
                pa = ps_h.tile([128, 512], F32, name="pa", tag="fps_h")
                pg = ps_h.tile([128, 512], F32, name="pg", tag="fps_h")
                for kt in range(KT):
                    nc.tensor.matmul(
                        pa[:], lhsT=win[:, kt, m * 128 : (m + 1) * 128],
                        rhs=tn2f[:, kt, :],
                        start=(kt == 0), stop=(kt == KT - 1),
                    )
                for kt in range(KT):
                    nc.tensor.matmul(
                        pg[:],
                        lhsT=win[:, kt, 2048 + m * 128 : 2048 + (m + 1) * 128],
                        rhs=tn2f[:, kt, :],
                        start=(kt == 0), stop=(kt == KT - 1),
                    )
                nc.scalar.activation(gl[:], pg[:], AF.Gelu)
                nc.vector.tensor_tensor(u[:, m, :], pa[:], gl[:], op=OP.mult)
            # ---- x += u @ Wout
            for j in range(4):
                for nh in range(2):
                    px = ps_xd.tile([128, 384], F32, name="fpx", tag="fps_xd")
                    for ktf in range(16):
                        nc.tensor.matmul(
                            px[:],
                            lhsT=u[:, ktf, j * 128 : (j + 1) * 128],
                            rhs=wout[:, ktf, nh * 384 : (nh + 1) * 384],
                            start=(ktf == 0),
                            stop=(ktf == 15),
                        )
                    xs = x_sb[:, ds(coff + j, 1), nh * 384 : (nh + 1) * 384]
                    xs = xs.squeeze(1)
                    nc.vector.tensor_tensor(xs, xs, px[:], op=OP.add)

        for _cv in range(4):
            chunk_body(_cv)


def _final_norm(nc, tc, x_sb, x_out):
    with ExitStack() as ctx:
        sp = ctx.enter_context(tc.tile_pool(name="fnsp", bufs=2))
        np_ = ctx.enter_context(tc.tile_pool(name="fnnp", bufs=2))
        for sv in range(NSEQ):
            off = sv * 2
            sq = sp.tile([128, 768], F32, name="fnsq", tag="fnsq")
            ss = np_.tile([128, 2], F32, name="fnss", tag="fnss")
            for j in range(2):
                nc.scalar.activation(
                    sq[:], x_sb[:, ds(off + j, 1), :].squeeze(1), AF.Square,
                    accum_out=ss[:, j : j + 1],
                )
            inv = np_.tile([128, 2], F32, name="fninv", tag="fninv")
            _emit_rsqrt(nc, np_, inv[:], ss[:], 1.0 / 768.0, 1e-6, 1e-30)
            out_t = sp.tile([128, 2, 768], F32, name="fnout", tag="fnout")
            for j in range(2):
                nc.vector.tensor_scalar_mul(
                    out_t[:, j, :], x_sb[:, ds(off + j, 1), :].squeeze(1),
                    inv[:, j : j + 1],
                )
            nc.sync.dma_start(
                x_out[ds(sv * 256, 256), :].rearrange("(j p) d -> p j d", p=128),
                out_t[:],
            )


# ---------------------------------------------------------------------------
# binding / execution
# ---------------------------------------------------------------------------


def _collect_io(nc):
    import jax

    in_names, out_names, out_avals = [], [], []
    pname = nc.partition_id_tensor.name if nc.partition_id_tensor else None
    for alloc in nc.m.functions[0].allocations:
        if not isinstance(alloc, mybir.MemoryLocationSet):
            continue
        if not alloc.memorylocations:
            continue
        name = alloc.memorylocations[0].name
        if alloc.kind == "ExternalInput" and name != pname:
            in_names.append(name)
        elif alloc.kind == "ExternalOutput":
            out_names.append(name)
            out_avals.append(
                jax.core.ShapedArray(
                    tuple(alloc.tensor_shape), mybir.dt.np(alloc.dtype)
                )
            )
    return in_names, out_names, out_avals


_PIPE = None


def _tok_to_bt(tokens):
    """(B, T, S, D) -> global (NC*B*TL, S, D): rows (c, b, tl), t = 4c+tl."""
    return np.ascontiguousarray(
        tokens.transpose(1, 0, 2, 3)
        .reshape(NC, TL, B, S, DIM)
        .transpose(0, 2, 1, 3, 4)
    ).reshape(NC * B * TL, S, DIM)


def _out_to_full(out):
    """(NC*B*SL, T, D) rows (c, b, sl) with s=32c+sl -> (B, T, S, D)."""
    out = out.reshape(NC, B, SL, T, DIM).transpose(1, 3, 0, 2, 4)
    return out.reshape(B, T, S, DIM)


def _build_pipeline(inputs):
    devs = jax.devices()[:NC]
    mesh = Mesh(np.asarray(devs), ("core",))
    shard = NamedSharding(mesh, P("core"))

    nc = build_full()
    from concourse import bass2jax
    from concourse.bass2jax import _bass_exec_p

    in_names, out_names, out_avals = _collect_io(nc)
    bind_names = tuple(in_names + out_names)
    pid_name = nc.partition_id_tensor.name if nc.partition_id_tensor else None
    full_names = bind_names + ((pid_name,) if pid_name else ())

    def bass_body(*args):
        ops = list(args)
        if pid_name is not None:
            ops.append(bass2jax.partition_id_tensor())
        outs = _bass_exec_p.bind(
            *ops,
            out_avals=tuple(out_avals),
            in_names=full_names,
            out_names=tuple(out_names),
            lowering_input_output_aliases=(),
            sim_require_finite=True,
            sim_require_nnan=True,
            nc=nc,
        )
        return tuple(outs)

    percore = {"x_in"} | set(out_names)
    in_specs = tuple(P("core") if n in percore else P() for n in bind_names)
    out_specs = (P("core"),) * len(out_names)
    nout = len(out_names)
    bass_jit = jax.jit(
        shard_map(bass_body, mesh=mesh, in_specs=in_specs,
                  out_specs=out_specs, check_rep=False),
        donate_argnums=tuple(range(len(bind_names) - nout, len(bind_names))),
    )

    pk = _pack_weights(inputs)
    repl = NamedSharding(mesh, P())
    wdev = {k: jax.device_put(v, repl) for k, v in pk.items()}

    zero_outs = jax.jit(
        lambda: tuple(
            jnp.zeros((NC * aval.shape[0],) + aval.shape[1:], aval.dtype)
            for aval in out_avals
        ),
        out_shardings=tuple(shard for _ in out_avals),
    )

    def run(tok_bt):
        tok = jax.device_put(tok_bt, shard).reshape(NC * NTOK, DIM)
        ops = []
        for nme in in_names:
            if nme == "x_in":
                ops.append(tok)
            else:
                ops.append(wdev[nme])
        outs = bass_jit(*ops, *zero_outs())
        return dict(zip(out_names, outs))

    run.nc = nc
    run.in_names = in_names
    run.out_names = out_names
    run.weights = pk
    return run


def kernel(**inputs):
    global _PIPE
    tokens = np.asarray(inputs["tokens"], dtype=np.float32)
    tok_bt = _tok_to_bt(tokens)

    if _PIPE is None:
        _PIPE = _build_pipeline(inputs)
    outs = _PIPE(jnp.asarray(tok_bt))
    out = np.asarray(jax.block_until_ready(outs["x_out"]))

    out = _out_to_full(out.reshape(NC * B * SL, T, DIM))
    out = out * np.asarray(inputs["final_norm_w"], np.float32)
    _PIPE.last_outs = outs
    return np.ascontiguousarray(out.astype(np.float32))

